# revision 29
# baseline (speedup 1.0000x reference)
"""Trainium2 Bass kernel for nn_Classifier (GNN edge-MLP link predictor).

Computes, for E candidate edges:
    out[e] = W2 . relu( x_nc[i0[e]] @ W1[:H] + x_pr[i1[e]] @ W1[H:] + b1 ) + b2

v5 strategy ("resident-A + gathered-B", 8 cores):
  v1 bottleneck: SWDGE descriptor generation on the GpSimd Q7 cluster
  saturates at ~2.6 ns/gathered-row across all 4 queues (~640us for
  250k rows/core).  v5 eliminates the a-side rows entirely:

  * Edges are range-sharded by i0: core k owns the ~125k edges whose
    i0 falls in its contiguous ~2500-row slice of the ncRNA table
    (multiplicity ~50 edges/node within the core).
  * The whole a-slice (2560 x 128 fp16 = 640KB) stays RESIDENT IN
    SBUF, written directly by the phase-1 activation (no DRAM round
    trip).  Node n's row lives at partition n%128, free col block
    (n//512)*4 + (n%512)//128 - exactly where the phase-1 psum leaves
    it.
  * Side A (i0): edges with occurrence rank occ < K=46 get slot
    (p, col = 104 + occ*20 + c) matching their a-row (p, c) in SBUF.
    The epilogue's DVE add reads the a-operand STRAIGHT from the
    resident slice (per-layer slices; 1-3 extra DVE instrs/tile) -
    zero descriptors, zero copies.  Leftover duplicates (occ >= K,
    <=13.3k) fill slot cols [0, 104) via two small dma_gathers from a
    DRAM copy of the slice, emitted first so they overlap the b-table
    build.  All index padding uses 0 (a valid row): trailing -1
    trimming desyncs SWDGE ring bookkeeping and crashes the device.
  * Side B (i1): per-tile dma_gather from the full b-table into the
    edge slots dictated by side A (131k descriptors vs 250k in v1).
  * Tables are stored row-PERMUTED: node n -> row phi(n) =
    (n//512)*512 + (n%128)*4 + (n%512)//128, making phase-1 writes
    contiguous per partition (1KB descriptors).
  * T=4096 tiles with deep, separate pools per pipeline stage so 4
    SWDGE queues stay fed (the v4 limiter was buffer-rotation
    latency, not Q7 throughput).

Output lands as [128, COLS] f32 slot grid per core; the host scatters
slots back to original edge positions.
"""

import numpy as np
import ml_dtypes

import concourse.bass as bass
import concourse.tile as tile
from concourse import bacc, mybir
from concourse import bass_utils

F32 = mybir.dt.float32
FP16 = mybir.dt.float16
BF16 = mybir.dt.bfloat16
I16 = mybir.dt.int16

N_CORES = 8
H = 128

E_TOTAL = 1_000_000
N_NODES = 20_000
NP = 20_480         # b table rows (full, padded to 40 chunks of 512)
WIDTH = 2_560       # a table slice rows (padded; 5 chunks of 512)
K_LAYERS = 46       # sweep layers
LCOLS = WIDTH // 128            # 20 cols per layer
SINGLE_BASE = 0
SINGLE_COLS = 104               # singles capacity 13312
SWEEP_BASE = SINGLE_COLS        # 104
TILE_COLS = 32
T = TILE_COLS * 128             # 4096 slots per tile
COLS = SWEEP_BASE + K_LAYERS * LCOLS  # 1024 = 32 tiles
N_TILES = COLS // TILE_COLS
E_CORE = E_TOTAL // N_CORES

add_op = mybir.AluOpType.add
mult_op = mybir.AluOpType.mult
ident = mybir.ActivationFunctionType.Identity
relu_fn = mybir.ActivationFunctionType.Relu


def _phi(n):
    """Table-row permutation making phase-1 writes contiguous."""
    return (n // 512) * 512 + (n % 128) * 4 + (n % 512) // 128


def _slot_of(n):
    """Resident a-slice position of node n: (partition, col-in-layer)."""
    return n % 128, (n // 512) * 4 + (n % 512) // 128


def _build(p_pos):
    nc = bacc.Bacc(
        "TRN2",
        target_bir_lowering=False,
        debug=False,
        num_devices=N_CORES,
        num_swdge_queues=4,
    )

    xt_pr = nc.dram_tensor("xt_pr", [H, NP], BF16, kind="ExternalInput").ap()
    xt_nc = nc.dram_tensor("xt_nc", [H, WIDTH], BF16, kind="ExternalInput").ap()
    w1nc = nc.dram_tensor("w1nc", [H, H], BF16, kind="ExternalInput").ap()
    w1pr = nc.dram_tensor("w1pr", [H, H], BF16, kind="ExternalInput").ap()
    b1r = nc.dram_tensor("b1r", [1, H], BF16, kind="ExternalInput").ap()
    b2 = nc.dram_tensor("b2", [128, 1], F32, kind="ExternalInput").ap()
    idxB = nc.dram_tensor("idxB", [16, COLS * 8], I16, kind="ExternalInput").ap()
    idxA = nc.dram_tensor("idxA", [16, SINGLE_COLS * 8], I16, kind="ExternalInput").ap()
    out = nc.dram_tensor("out", [128, COLS], F32, kind="ExternalOutput").ap()

    a_tbl = nc.dram_tensor("a_tbl", [WIDTH, H], FP16, kind="Internal").ap()
    b_tbl = nc.dram_tensor("b_tbl", [NP, H], FP16, kind="Internal").ap()

    with tile.TileContext(nc) as tc:
        with (
            tc.tile_pool(name="const", bufs=1) as cpool,
            tc.tile_pool(name="idx", bufs=1) as ipool,
            tc.tile_pool(name="x", bufs=6) as xpool,
            tc.tile_pool(name="ao", bufs=4) as apool,
            tc.tile_pool(name="gA", bufs=2) as gApool,
            tc.tile_pool(name="gB", bufs=7) as gBpool,
            tc.tile_pool(name="h", bufs=5) as hpool,
            tc.tile_pool(name="hr", bufs=4) as rpool,
            tc.tile_pool(name="stage", bufs=4) as spool,
            tc.tile_pool(name="ps", bufs=6, space="PSUM") as pspool,
        ):
            # ---- constants ----
            w1nc_sb = cpool.tile([H, H], BF16, tag="w1nc")
            nc.sync.dma_start(w1nc_sb[:], w1nc[:])
            w1pr_sb = cpool.tile([H, H], BF16, tag="w1pr")
            nc.sync.dma_start(w1pr_sb[:], w1pr[:])
            b1_row = cpool.tile([1, H], BF16, tag="b1row")
            nc.sync.dma_start(b1_row[:], b1r[:])
            ones_sb = cpool.tile([1, H], BF16, tag="ones")
            nc.vector.memset(ones_sb[:], 1.0)
            b2_rep = cpool.tile([128, 1], F32, tag="b2rep")
            nc.sync.dma_start(b2_rep[:], b2[:])

            # resident a-slice [128, WIDTH] fp16 (5KB/partition)
            a_sb = cpool.tile([128, WIDTH], FP16, tag="a_sb")

            # ---- indices (wrapped by 16, replicated to 8 groups) ----
            idxB_sb = ipool.tile([128, COLS * 8], I16, tag="idxB")
            idxA_sb = ipool.tile([128, SINGLE_COLS * 8], I16, tag="idxA")
            for k in range(8):
                nc.sync.dma_start(idxA_sb[16 * k : 16 * (k + 1), :], idxA[:])

            # ---- phase 1 ----
            # order: b chunks 0-15 (unblocks the i1-sorted singles B-gathers
            # on the b_tbl prefix), then the a slice (unblocks A-singles and
            # the resident adds), then b chunks 16-39.
            a_view = a_tbl.rearrange("(c p k) f -> p c k f", p=128, k=4)
            b_view = b_tbl.rearrange("(c p k) f -> p c k f", p=128, k=4)

            def b_chunk(c):
                sl = slice(c * 512, (c + 1) * 512)
                xc = xpool.tile([H, 512], BF16, tag="xc")
                nc.sync.dma_start(xc[:], xt_pr[:, sl])
                ps = pspool.tile([128, 512], F32, tag="ps")
                for k in range(4):
                    ks = slice(k * 128, (k + 1) * 128)
                    nc.tensor.matmul(
                        ps[:, ks], xc[:, ks], w1pr_sb[:], start=True, stop=True
                    )
                ao = apool.tile([128, 512], FP16, tag="ao")
                nc.scalar.activation(ao[:], ps[:], ident)
                nc.sync.dma_start(
                    b_view[:, c, :, :],
                    ao[:].rearrange("p (k f) -> p k f", k=4),
                )

            for c in range(WIDTH // 512):
                sl = slice(c * 512, (c + 1) * 512)
                xc = xpool.tile([H, 512], BF16, tag="xc")
                nc.sync.dma_start(xc[:], xt_nc[:, sl])
                ps = pspool.tile([128, 512], F32, tag="ps")
                for k in range(4):
                    ks = slice(k * 128, (k + 1) * 128)
                    nc.tensor.matmul(
                        ps[:, ks], ones_sb[:], b1_row[:], start=True, stop=False
                    )
                    nc.tensor.matmul(
                        ps[:, ks], xc[:, ks], w1nc_sb[:], start=False, stop=True
                    )
                # straight into the resident slice (node c*512+k*128+p ->
                # partition p, cols (c*4+k)*128 + f == contiguous c*512 block)
                nc.scalar.activation(a_sb[:, sl], ps[:], ident)
                # DRAM copy for the singles gather
                nc.sync.dma_start(
                    a_view[:, c, :, :],
                    a_sb[:, sl].rearrange("p (k f) -> p k f", k=4),
                )
            for k in range(8):
                nc.sync.dma_start(idxB_sb[16 * k : 16 * (k + 1), :], idxB[:])
            for c in range(NP // 512):
                b_chunk(c)

            # ---- phase 2: per-tile gather / MLP ----
            qn = 0
            for t in range(N_TILES):
                c1, c2 = t * TILE_COLS, (t + 1) * TILE_COLS

                # singles part (cols [0, SWEEP_BASE)) -> gA tile
                gAt = None
                s1, s2 = c1, min(c2, SWEEP_BASE)
                if s1 < s2:
                    gAt = gApool.tile([128, T], FP16, tag="gA")
                    n = (s2 - s1) * 128
                    o = (s1 - SINGLE_BASE) * 8
                    nc.gpsimd.dma_gather(
                        gAt[:, : (s2 - s1) * H].rearrange(
                            "p (c f) -> p c f", f=H
                        ),
                        a_tbl,
                        idxA_sb[:, o : o + n // 16],
                        n,
                        n,
                        H,
                        transpose=False,
                        single_packet=False,
                        queue_num=qn % 4,
                    )
                    qn += 1

                # i1-sorted singles let the first two tiles depend only on a
                # prefix of b_tbl (host asserts the index bounds hold)
                b_src = b_tbl
                if t == 0:
                    b_src = b_tbl[0:8192]
                elif t == 1:
                    b_src = b_tbl[0:15872]
                # B-gather split into two 2048-idx halves on adjacent queues:
                # halves the latency the first add waits on and the work
                # blocked behind any stalled gather.
                gBt = gBpool.tile([128, T], FP16, tag="gB")
                hcw = TILE_COLS // 2
                for hf in range(2):
                    nc.gpsimd.dma_gather(
                        gBt[:, hf * hcw * H : (hf + 1) * hcw * H].rearrange(
                            "p (c f) -> p c f", f=H
                        ),
                        b_src,
                        idxB_sb[:, (c1 + hf * hcw) * 8 : (c1 + (hf + 1) * hcw) * 8],
                        T // 2,
                        T // 2,
                        H,
                        transpose=False,
                        single_packet=False,
                        queue_num=qn % 4,
                    )
                    qn += 1

                # add: a-operand from gA tile (singles) and/or resident
                # slice; parts split at layer and gather-half boundaries
                h = hpool.tile([128, T], FP16, tag="h")
                parts = []
                if s1 < s2:
                    parts.append((s1, s2, None))
                lo = max(c1, SWEEP_BASE)
                while lo < c2:
                    lyr = (lo - SWEEP_BASE) // LCOLS
                    hi = min(c2, SWEEP_BASE + (lyr + 1) * LCOLS)
                    parts.append((lo, hi, lyr))
                    lo = hi
                mid = c1 + hcw
                split = []
                for lo, hi, lyr in parts:
                    if lo < mid < hi:
                        split += [(lo, mid, lyr), (mid, hi, lyr)]
                    else:
                        split.append((lo, hi, lyr))
                for lo, hi, lyr in split:
                    d1, d2 = (lo - c1) * H, (hi - c1) * H
                    if lyr is None:
                        a_op = gAt[:, d1:d2]
                    else:
                        lb = SWEEP_BASE + lyr * LCOLS
                        a_op = a_sb[:, (lo - lb) * H : (hi - lb) * H]
                    nc.vector.tensor_tensor(
                        h[:, d1:d2], a_op, gBt[:, d1:d2], add_op
                    )

                # |w2| is folded into W1/b1 on the host, features permuted
                # so w2>0 comes first: out = sum(pos relu) - sum(neg relu)
                hr = rpool.tile([128, T], FP16, tag="hr")
                nc.scalar.activation(hr[:], h[:], relu_fn)
                hrv = hr[:].rearrange("p (g f) -> p g f", f=H)
                red = spool.tile([128, TILE_COLS], F32, tag="red")
                nc.vector.tensor_reduce(
                    red[:], hrv[:, :, 0:p_pos], mybir.AxisListType.X, add_op
                )
                redn = spool.tile([128, TILE_COLS], F32, tag="redn")
                nc.vector.tensor_reduce(
                    redn[:], hrv[:, :, p_pos:H], mybir.AxisListType.X, add_op
                )
                diff = spool.tile([128, TILE_COLS], F32, tag="diff")
                nc.vector.tensor_tensor(
                    diff[:], red[:], redn[:], mybir.AluOpType.subtract
                )
                stage = spool.tile([128, TILE_COLS], F32, tag="stage")
                nc.scalar.activation(stage[:], diff[:], ident, bias=b2_rep[:])
                nc.sync.dma_start(out[:, c1:c2], stage[:])

    nc.compile()
    return nc


# ---------------------------------------------------------------------------
# Host-side wrapper
# ---------------------------------------------------------------------------

_CACHE: dict = {}


def _get_program(p_pos):
    if p_pos not in _CACHE:
        _CACHE[p_pos] = _build(p_pos)
    return _CACHE[p_pos]


def _wrap16(flat: np.ndarray) -> np.ndarray:
    """int16 [16, n//16] with element i at [i % 16, i // 16]."""
    n = flat.shape[0]
    return np.ascontiguousarray(flat.reshape(n // 16, 16).T)


def kernel(
    x_ncRNA: np.ndarray,
    x_Protein: np.ndarray,
    edge_label_index: np.ndarray,
    W1: np.ndarray,
    b1: np.ndarray,
    W2: np.ndarray,
    b2: np.ndarray,
    _trace: bool = False,
) -> np.ndarray:
    E = edge_label_index.shape[1]
    n_nodes = x_ncRNA.shape[0]
    assert E == E_TOTAL and n_nodes == N_NODES

    i0 = np.asarray(edge_label_index[0]).astype(np.int64)
    i1 = np.asarray(edge_label_index[1]).astype(np.int64)

    # fold |w2| into W1/b1 columns; permute features so w2>0 comes first
    w2v = np.asarray(W2[:, 0], np.float64)
    forder = np.argsort(w2v <= 0, kind="stable")
    p_pos = int((w2v > 0).sum())
    assert 0 < p_pos < H
    scale = np.abs(w2v[forder])

    nc = _get_program(p_pos)

    # shared weight prep
    x_pr_t = np.zeros((H, NP), ml_dtypes.bfloat16)
    x_pr_t[:, :n_nodes] = x_Protein.T.astype(ml_dtypes.bfloat16)
    w1nc = np.ascontiguousarray(
        (W1[:H][:, forder] * scale).astype(ml_dtypes.bfloat16)
    )
    w1pr = np.ascontiguousarray(
        (W1[H:][:, forder] * scale).astype(ml_dtypes.bfloat16)
    )
    b1r = np.ascontiguousarray(
        (b1[forder] * scale).astype(ml_dtypes.bfloat16).reshape(1, H)
    )
    b2_ = np.ascontiguousarray(
        np.broadcast_to(b2.reshape(1, 1), (128, 1)).astype(np.float32)
    )

    order = np.argsort(i0, kind="stable")

    in_maps = []
    slot_p = np.empty(E, np.int64)
    slot_c = np.empty(E, np.int64)
    for c in range(N_CORES):
        sel = order[c * E_CORE : (c + 1) * E_CORE]
        vals = i0[sel]
        v_lo = int(vals[0])
        width = int(vals[-1]) - v_lo + 1
        assert width <= WIDTH, f"core {c}: slice width {width} > {WIDTH}"
        vloc = vals - v_lo
        first = np.searchsorted(vals, vals, side="left")
        occ = np.arange(E_CORE) - first

        swept = occ < K_LAYERS
        n_single = int((~swept).sum())
        assert n_single <= SINGLE_COLS * 128, f"core {c}: {n_single} singles"

        sp, sc = _slot_of(vloc)
        p_arr = np.empty(E_CORE, np.int64)
        col_arr = np.empty(E_CORE, np.int64)
        p_arr[swept] = sp[swept]
        col_arr[swept] = SWEEP_BASE + occ[swept] * LCOLS + sc[swept]
        # singles sorted by i1 so the first tiles only touch a b_tbl prefix
        sidx = np.where(~swept)[0]
        sidx = sidx[np.argsort(i1[sel][sidx], kind="stable")]
        ks = np.arange(n_single)
        p_arr[sidx] = ks % 128
        col_arr[sidx] = SINGLE_BASE + ks // 128
        s_i1 = i1[sel][sidx]
        if n_single > 4096:
            assert s_i1[4095] < 8192, f"core {c}: tile0 b-prefix violated"
        if n_single > 8192:
            assert s_i1[8191] < 15872, f"core {c}: tile1 b-prefix violated"
        slot_p[c * E_CORE : (c + 1) * E_CORE] = p_arr
        slot_c[c * E_CORE : (c + 1) * E_CORE] = col_arr

        idxB_slot = np.zeros((128, COLS), np.int16)
        idxB_slot[p_arr, col_arr] = _phi(i1[sel]).astype(np.int16)
        flatB = idxB_slot.T.reshape(-1)

        flatA = np.zeros(SINGLE_COLS * 128, np.int16)
        flatA[:n_single] = _phi(vloc[sidx]).astype(np.int16)

        x_nc_t = np.zeros((H, WIDTH), ml_dtypes.bfloat16)
        x_nc_t[:, :width] = x_ncRNA[v_lo : v_lo + width].T.astype(
            ml_dtypes.bfloat16
        )

        in_maps.append(
            {
                "xt_pr": x_pr_t,
                "xt_nc": np.ascontiguousarray(x_nc_t),
                "w1nc": w1nc,
                "w1pr": w1pr,
                "b1r": b1r,
                "b2": b2_,
                "idxB": _wrap16(flatB),
                "idxA": _wrap16(flatA),
            }
        )

    res = bass_utils.run_bass_kernel_spmd(
        nc, in_maps, core_ids=list(range(N_CORES)), trace=_trace
    )

    out = np.empty(E, np.float32)
    for c in range(N_CORES):
        grid = res.results[c]["out"]  # [128, COLS]
        out[order[c * E_CORE : (c + 1) * E_CORE]] = grid[
            slot_p[c * E_CORE : (c + 1) * E_CORE],
            slot_c[c * E_CORE : (c + 1) * E_CORE],
        ]
    kernel._last_results = res
    return out
